# revision 6
# baseline (speedup 1.0000x reference)
"""Trainium2 Bass kernel for the custom LSTM problem.

Strategy: single-core, collective-free. Per-step AllGather collectives cost
~13ms each under this runtime, so all cross-core schemes lose; one core runs
the full recurrence instead.

Phase A: build a projected embedding table P[v, :] = emb[v] @ Wi.T + b
([VPAD, 4G] bf16, in internal DRAM) — one 134-GFLOP GEMM instead of 275
GFLOP of per-step input projections.
Phase B: For_i hardware loop over T steps. Per step: dma_gather the 128
token rows of P (that step's xp, bias included), 64 N=512 matmuls for
h @ Wh.T accumulated on top in PSUM is not possible (PSUM too small), so
z = psum(h@Wh.T) + xp via DVE adds; gate order per 2048-wide half is
[i|f|o|g] so one Sigmoid covers 1536 cols and one Tanh the g block.
h is re-transposed each step with PE transposes (hT is the next step's
stationary operand). Hardware loop keeps the program ~500 instructions,
so bass build + NEFF compile stay fast.

Gate column permutation (host side): half h in {0,1}, order [i f o g]
within each half; h-dim (Wh columns, c, h state) stays in natural order.
"""

import os
import numpy as np
import ml_dtypes

import concourse.bass as bass
import concourse.mybir as mybir
import concourse.tile as tile
from concourse import bacc
from concourse.bass import ds
from concourse.bass_utils import run_bass_kernel_spmd
from concourse.masks import make_identity

V, E, H, B, T_FULL, O = 32000, 512, 1024, 128, 512, 1
VPAD = 32768
G4 = 4 * H          # 4096 gate columns
PAD_IDX = 0
NVT = VPAD // 128   # 256 vocab tiles

f32 = mybir.dt.float32
bf16 = mybir.dt.bfloat16
i16 = mybir.dt.int16
ACT = mybir.ActivationFunctionType

LAST_EXEC_NS = None
_built = {}


def _build(t_steps):
    if t_steps in _built:
        return _built[t_steps]
    assert t_steps % 2 == 0

    nc = bacc.Bacc("TRN2", target_bir_lowering=False, debug=False,
                   num_devices=1)

    embT_d = nc.dram_tensor("embT", [128, 4 * VPAD], bf16, kind="ExternalInput")
    wiT_d = nc.dram_tensor("wiT", [128, 4 * G4], bf16, kind="ExternalInput")
    whT_d = nc.dram_tensor("whT", [128, 8 * G4], bf16, kind="ExternalInput")
    brow_d = nc.dram_tensor("brow", [128, G4], bf16, kind="ExternalInput")
    idx_d = nc.dram_tensor("idx16", [128, 8 * t_steps + 8], i16,
                           kind="ExternalInput")
    mask_d = nc.dram_tensor("maskv", [B, t_steps], f32, kind="ExternalInput")
    fcw_d = nc.dram_tensor("fcw", [128, 8], f32, kind="ExternalInput")
    fcb_d = nc.dram_tensor("fcb", [1, B], f32, kind="ExternalInput")
    y_d = nc.dram_tensor("y", [1, B], f32, kind="ExternalOutput")

    with tile.TileContext(nc) as tc:
        with (
            tc.tile_pool(name="const", bufs=1) as constp,
            tc.tile_pool(name="state", bufs=1) as state,
            tc.tile_pool(name="work", bufs=2) as work,
            tc.tile_pool(name="emb", bufs=3) as embp,
            tc.tile_pool(name="zps", bufs=5, space="PSUM") as zps,
            tc.tile_pool(name="tps", bufs=2, space="PSUM") as tps,
            tc.tile_pool(name="fps", bufs=1, space="PSUM") as fpsp,
            tc.tile_pool(name="ptab", bufs=1, space="DRAM") as ptab,
        ):
            # ---- constants ----
            wiT_sb = constp.tile([128, 4, G4], bf16, name="wiT_sb")
            nc.sync.dma_start(
                wiT_sb[:], wiT_d.ap().rearrange("p (ko n) -> p ko n", ko=4))
            whT_sb = constp.tile([128, 8, G4], bf16, name="whT_sb")
            nc.sync.dma_start(
                whT_sb[:], whT_d.ap().rearrange("p (k n) -> p k n", k=8))
            brow_sb = constp.tile([128, G4], bf16, name="brow_sb")
            nc.sync.dma_start(brow_sb[:], brow_d.ap())
            idx_sb = constp.tile([128, 8 * t_steps + 8], i16, name="idx_sb")
            nc.sync.dma_start(idx_sb[:], idx_d.ap())
            mask_sb = constp.tile([B, t_steps], f32, name="mask_sb")
            nc.sync.dma_start(mask_sb[:], mask_d.ap())
            fcw_sb = constp.tile([128, 8], f32, name="fcw_sb")
            nc.sync.dma_start(fcw_sb[:], fcw_d.ap())
            fcb_sb = constp.tile([1, B], f32, name="fcb_sb")
            nc.sync.dma_start(fcb_sb[:], fcb_d.ap())
            ident = constp.tile([128, 128], f32, name="ident")
            make_identity(nc, ident[:])

            P_t = ptab.tile([VPAD, G4], bf16, name="P_t")

            embT_ap = embT_d.ap().rearrange("p (ko v) -> p ko v", ko=4)

            # ---- phase A: P = embT.T @ WiT + b ----
            def vtile(vb):
                et = embp.tile([128, 4, 128], bf16, tag="et", name="et")
                nc.sync.dma_start(et[:], embT_ap[:, :, ds(vb, 128)])
                pt = work.tile([128, G4], bf16, tag="pt", name="pt")
                for n in range(8):
                    ps = zps.tile([128, 512], f32, tag="z", name=f"za{n}")
                    for ke in range(4):
                        nc.tensor.matmul(
                            ps[:], et[:, ke, :],
                            wiT_sb[:, ke, 512 * n:512 * (n + 1)],
                            start=(ke == 0), stop=(ke == 3))
                    nc.vector.tensor_add(
                        pt[:, 512 * n:512 * (n + 1)], ps[:],
                        brow_sb[:, 512 * n:512 * (n + 1)])
                nc.sync.dma_start(P_t[ds(vb, 128), :], pt[:])

            with tc.For_i(0, VPAD, 256) as vb:
                vtile(vb)
                vtile(vb + 128)

            # ---- phase B state ----
            c_t = state.tile([B, H], f32, name="c_t")
            nc.vector.memset(c_t[:], 0.0)
            oacc = state.tile([B, H], f32, name="oacc")
            nc.vector.memset(oacc[:], 0.0)
            hTa = state.tile([128, 512], bf16, name="hTa")
            nc.vector.memset(hTa[:], 0.0)
            hTb = state.tile([128, 512], bf16, name="hTb")
            nc.vector.memset(hTb[:], 0.0)
            ping = state.tile([128, 1, G4], bf16, name="ping")
            pong = state.tile([128, 1, G4], bf16, name="pong")

            def gather(step_expr, dst):
                nc.gpsimd.dma_gather(
                    out_ap=dst[:], in_ap=P_t[:],
                    idxs_ap=idx_sb[:, ds(step_expr * 8, 8)],
                    num_idxs=128, num_idxs_reg=128, elem_size=G4)

            gather(0, ping)

            def step(t_expr, xp):
                # recurrence matmuls for both halves (read OLD hTa/hTb)
                zt = []
                for half in range(2):
                    for n in range(4):
                        g0 = 2048 * half + 512 * n
                        ps = zps.tile([128, 512], f32, tag="z", name="z")
                        for k in range(8):
                            src = hTa if k < 4 else hTb
                            nc.tensor.matmul(
                                ps[:], src[:, 128 * (k % 4):128 * (k % 4 + 1)],
                                whT_sb[:, k, g0:g0 + 512],
                                start=(k == 0), stop=(k == 7))
                        zt.append(ps)
                # gate tails per half (write NEW hTa/hTb)
                for half in range(2):
                    zf = work.tile([B, 2048], f32, tag="zf", name="zf")
                    for n in range(4):
                        nc.vector.tensor_add(
                            zf[:, 512 * n:512 * (n + 1)], zt[4 * half + n][:],
                            xp[:, 0, 2048 * half + 512 * n:
                               2048 * half + 512 * (n + 1)])
                    s = work.tile([B, 2048], f32, tag="s", name="s")
                    nc.scalar.activation(s[:, 0:1536], zf[:, 0:1536],
                                         ACT.Sigmoid)
                    nc.scalar.activation(s[:, 1536:2048], zf[:, 1536:2048],
                                         ACT.Tanh)
                    hs = slice(512 * half, 512 * (half + 1))
                    ig = work.tile([B, 512], f32, tag="ig", name="ig")
                    nc.vector.tensor_mul(ig[:], s[:, 0:512], s[:, 1536:2048])
                    cf = work.tile([B, 512], f32, tag="cf", name="cf")
                    nc.vector.tensor_mul(cf[:], c_t[:, hs], s[:, 512:1024])
                    nc.vector.tensor_add(c_t[:, hs], cf[:], ig[:])
                    thc = work.tile([B, 512], f32, tag="thc", name="thc")
                    nc.scalar.activation(thc[:], c_t[:, hs], ACT.Tanh)
                    hh = work.tile([B, 512], f32, tag="hh", name="hh")
                    nc.vector.tensor_mul(hh[:], s[:, 1024:1536], thc[:])
                    nc.vector.scalar_tensor_tensor(
                        oacc[:, hs], hh[:], mask_sb[:, ds(t_expr, 1)],
                        oacc[:, hs],
                        mybir.AluOpType.mult, mybir.AluOpType.add)
                    tp = tps.tile([128, 512], f32, tag="tp", name="tp")
                    for j in range(4):
                        nc.tensor.transpose(tp[:, 128 * j:128 * (j + 1)],
                                            hh[:, 128 * j:128 * (j + 1)],
                                            ident[:])
                    nc.vector.tensor_copy((hTa if half == 0 else hTb)[:],
                                          tp[:])

            with tc.For_i(0, t_steps, 2) as tt:
                gather(tt + 1, pong)
                step(tt, ping)
                gather(tt + 2, ping)
                step(tt + 1, pong)

            # ---- final fc ----
            fps = fpsp.tile([1, B], f32, tag="fps", name="fps")
            for k4 in range(2):
                tpo = tps.tile([128, 512], f32, tag="tp", name=f"tpo{k4}")
                for j in range(4):
                    kk = 4 * k4 + j
                    nc.tensor.transpose(tpo[:, 128 * j:128 * (j + 1)],
                                        oacc[:, 128 * kk:128 * (kk + 1)],
                                        ident[:])
                oT = work.tile([128, 512], f32, tag="oT", name=f"oT{k4}")
                nc.vector.tensor_copy(oT[:], tpo[:])
                for j in range(4):
                    kk = 4 * k4 + j
                    nc.tensor.matmul(fps[:], fcw_sb[:, kk:kk + 1],
                                     oT[:, 128 * j:128 * (j + 1)],
                                     start=(kk == 0), stop=(kk == 7))
            ysb = work.tile([1, B], f32, tag="ysb", name="ysb")
            nc.vector.tensor_add(ysb[:], fps[:], fcb_sb[:])
            nc.sync.dma_start(y_d.ap(), ysb[:])

    nc.compile()
    _built[t_steps] = nc
    return nc


def _prep_inputs(x, lengths, emb, W_ii, W_hi, b_i, W_if, W_hf, b_f,
                 W_ig, W_hg, b_g, W_io, W_ho, b_o, fc_w, fc_b, t_steps):
    x = np.asarray(x).astype(np.int64)[:, :t_steps]
    lengths = np.asarray(lengths).astype(np.int64)
    emb = np.asarray(emb, dtype=np.float32).copy()
    emb[PAD_IDX] = 0.0

    # embT[p, ko, v] = emb[v, 128*ko + p]
    embT = np.zeros((128, 4, VPAD), dtype=ml_dtypes.bfloat16)
    embT[:, :, :V] = np.ascontiguousarray(
        emb.T.reshape(4, 128, V).transpose(1, 0, 2)).astype(ml_dtypes.bfloat16)

    # gate column permutation: [half][i f o g] x 512
    def permute_rows(Wi_, Wf_, Wg_, Wo_):
        blocks = []
        for half in range(2):
            hs = slice(512 * half, 512 * (half + 1))
            blocks += [np.asarray(Wi_)[hs], np.asarray(Wf_)[hs],
                       np.asarray(Wo_)[hs], np.asarray(Wg_)[hs]]
        return np.concatenate(blocks, axis=0)

    Wi_p = permute_rows(W_ii, W_if, W_ig, W_io)      # [4096, 512]
    Wh_p = permute_rows(W_hi, W_hf, W_hg, W_ho)      # [4096, 1024]
    b_p = permute_rows(b_i[:, None], b_f[:, None], b_g[:, None],
                       b_o[:, None]).ravel()         # [4096]

    # wiT[p, ke, g] = Wi_p[g, 128*ke + p]
    wiT = np.ascontiguousarray(
        Wi_p.T.reshape(4, 128, G4).transpose(1, 0, 2)).astype(
            ml_dtypes.bfloat16)
    whT = np.ascontiguousarray(
        Wh_p.T.reshape(8, 128, G4).transpose(1, 0, 2)).astype(
            ml_dtypes.bfloat16)
    brow = np.tile(b_p.astype(ml_dtypes.bfloat16)[None, :], (128, 1))

    # t-major token ids, SWDGE layout: per step a [16, 8] block, tiled x8
    xt = np.ascontiguousarray(x.T).astype(np.int16)     # [t, B]
    blocks = [xt[t].reshape(8, 16).T for t in range(t_steps)]
    blocks.append(np.zeros((16, 8), np.int16))          # overrun pad step
    idx16 = np.tile(np.concatenate(blocks, axis=1), (8, 1))

    maskv = (lengths[:, None] == (np.arange(t_steps)[None, :] + 1)).astype(
        np.float32)

    fc_w = np.asarray(fc_w, dtype=np.float32).reshape(O, H)
    fcw = np.ascontiguousarray(fc_w[0].reshape(8, 128).T)
    fcb = np.full((1, B), np.asarray(fc_b, np.float32).reshape(O)[0],
                  dtype=np.float32)

    return [{
        "embT": embT.reshape(128, 4 * VPAD),
        "wiT": wiT.reshape(128, 4 * G4),
        "whT": whT.reshape(128, 8 * G4),
        "brow": brow,
        "idx16": idx16,
        "maskv": maskv,
        "fcw": fcw,
        "fcb": fcb,
    }]


def kernel(**inputs):
    global LAST_EXEC_NS
    t_steps = int(os.environ.get("KERNEL_T", T_FULL))
    nc = _build(t_steps)
    in_maps = _prep_inputs(t_steps=t_steps, **inputs)
    res = run_bass_kernel_spmd(nc, in_maps, core_ids=[0])
    LAST_EXEC_NS = res.exec_time_ns
    y = np.asarray(res.results[0]["y"], dtype=np.float32).reshape(B)
    return y.reshape(B, O)


# revision 7
# speedup vs baseline: 4293.7941x; 4293.7941x over previous
"""Trainium2 Bass kernel for the custom LSTM problem.

Strategy: single-core, collective-free. Per-step AllGather collectives cost
~13ms each under this runtime, so all cross-core schemes lose; one core runs
the full recurrence instead.

Phase A: build a projected embedding table P[v, :] = emb[v] @ Wi.T + b
([VPAD, 4G] bf16, in internal DRAM) — one 134-GFLOP GEMM instead of 275
GFLOP of per-step input projections.
Phase B: For_i hardware loop over T steps. Per step: dma_gather the 128
token rows of P (that step's xp, bias included), 64 N=512 matmuls for
h @ Wh.T accumulated on top in PSUM is not possible (PSUM too small), so
z = psum(h@Wh.T) + xp via DVE adds; gate order per 2048-wide half is
[i|f|o|g] so one Sigmoid covers 1536 cols and one Tanh the g block.
h is re-transposed each step with PE transposes (hT is the next step's
stationary operand). Hardware loop keeps the program ~500 instructions,
so bass build + NEFF compile stay fast.

Gate column permutation (host side): half h in {0,1}, order [i f o g]
within each half; h-dim (Wh columns, c, h state) stays in natural order.
"""

import os
import numpy as np
import ml_dtypes

import concourse.bass as bass
import concourse.mybir as mybir
import concourse.tile as tile
from concourse import bacc
from concourse.bass import ds
from concourse.bass_utils import run_bass_kernel_spmd
from concourse.masks import make_identity

V, E, H, B, T_FULL, O = 32000, 512, 1024, 128, 512, 1
VPAD = 32768
G4 = 4 * H          # 4096 gate columns
PAD_IDX = 0
NVT = VPAD // 128   # 256 vocab tiles

f32 = mybir.dt.float32
bf16 = mybir.dt.bfloat16
i16 = mybir.dt.int16
ACT = mybir.ActivationFunctionType

LAST_EXEC_NS = None
_built = {}


def _build(t_steps):
    if t_steps in _built:
        return _built[t_steps]
    assert t_steps % 2 == 0

    nc = bacc.Bacc("TRN2", target_bir_lowering=False, debug=False,
                   num_devices=1)

    embT_d = nc.dram_tensor("embT", [128, 4 * VPAD], bf16, kind="ExternalInput")
    wiT_d = nc.dram_tensor("wiT", [128, 4 * G4], bf16, kind="ExternalInput")
    whT_d = nc.dram_tensor("whT", [128, 8 * G4], bf16, kind="ExternalInput")
    brow_d = nc.dram_tensor("brow", [128, G4], bf16, kind="ExternalInput")
    idx_d = nc.dram_tensor("idx16", [128, 8 * t_steps + 8], i16,
                           kind="ExternalInput")
    mask_d = nc.dram_tensor("maskv", [B, t_steps], f32, kind="ExternalInput")
    fcw_d = nc.dram_tensor("fcw", [128, 8], f32, kind="ExternalInput")
    fcb_d = nc.dram_tensor("fcb", [1, B], f32, kind="ExternalInput")
    y_d = nc.dram_tensor("y", [1, B], f32, kind="ExternalOutput")

    with tile.TileContext(nc) as tc:
        with (
            tc.tile_pool(name="const", bufs=1) as constp,
            tc.tile_pool(name="state", bufs=1) as state,
            tc.tile_pool(name="work", bufs=2) as work,
            tc.tile_pool(name="emb", bufs=3) as embp,
            tc.tile_pool(name="ptcp", bufs=1) as ptcp,
            tc.tile_pool(name="zps", bufs=5, space="PSUM") as zps,
            tc.tile_pool(name="tps", bufs=2, space="PSUM") as tps,
            tc.tile_pool(name="fps", bufs=1, space="PSUM") as fpsp,
            tc.tile_pool(name="ptab", bufs=1, space="DRAM") as ptab,
        ):
            # ---- constants ----
            wiT_sb = constp.tile([128, 4, G4], bf16, name="wiT_sb")
            nc.sync.dma_start(
                wiT_sb[:], wiT_d.ap().rearrange("p (ko n) -> p ko n", ko=4))
            whT_sb = constp.tile([128, 8, G4], bf16, name="whT_sb")
            nc.sync.dma_start(
                whT_sb[:], whT_d.ap().rearrange("p (k n) -> p k n", k=8))
            brow_sb = constp.tile([128, G4], bf16, name="brow_sb")
            nc.sync.dma_start(brow_sb[:], brow_d.ap())
            idx_sb = constp.tile([128, 8 * t_steps + 8], i16, name="idx_sb")
            nc.sync.dma_start(idx_sb[:], idx_d.ap())
            mask_sb = constp.tile([B, t_steps], f32, name="mask_sb")
            nc.sync.dma_start(mask_sb[:], mask_d.ap())
            fcw_sb = constp.tile([128, 8], f32, name="fcw_sb")
            nc.sync.dma_start(fcw_sb[:], fcw_d.ap())
            fcb_sb = constp.tile([1, B], f32, name="fcb_sb")
            nc.sync.dma_start(fcb_sb[:], fcb_d.ap())
            ident = constp.tile([128, 128], f32, name="ident")
            make_identity(nc, ident[:])

            P_t = ptab.tile([VPAD, G4], bf16, name="P_t")

            embT_ap = embT_d.ap().rearrange("p (ko v) -> p ko v", ko=4)

            # ---- phase A: P = embT.T @ WiT + b ----
            def vtile(vb):
                et = embp.tile([128, 4, 128], bf16, tag="et", name="et")
                nc.sync.dma_start(et[:], embT_ap[:, :, ds(vb, 128)])
                pt = ptcp.tile([128, G4], bf16, tag="pt", name="pt")
                for n in range(8):
                    ps = zps.tile([128, 512], f32, tag="z", name=f"za{n}")
                    for ke in range(4):
                        nc.tensor.matmul(
                            ps[:], et[:, ke, :],
                            wiT_sb[:, ke, 512 * n:512 * (n + 1)],
                            start=(ke == 0), stop=(ke == 3))
                    nc.vector.tensor_add(
                        pt[:, 512 * n:512 * (n + 1)], ps[:],
                        brow_sb[:, 512 * n:512 * (n + 1)])
                nc.sync.dma_start(P_t[ds(vb, 128), :], pt[:])

            with tc.For_i(0, VPAD, 256) as vb:
                vtile(vb)
                vtile(vb + 128)

            # ---- phase B state ----
            c_t = state.tile([B, H], f32, name="c_t")
            nc.vector.memset(c_t[:], 0.0)
            oacc = state.tile([B, H], f32, name="oacc")
            nc.vector.memset(oacc[:], 0.0)
            hTa = state.tile([128, 512], bf16, name="hTa")
            nc.vector.memset(hTa[:], 0.0)
            hTb = state.tile([128, 512], bf16, name="hTb")
            nc.vector.memset(hTb[:], 0.0)
            ping = state.tile([128, 1, G4], bf16, name="ping")
            pong = state.tile([128, 1, G4], bf16, name="pong")

            def gather(step_expr, dst):
                nc.gpsimd.dma_gather(
                    out_ap=dst[:], in_ap=P_t[:],
                    idxs_ap=idx_sb[:, ds(step_expr * 8, 8)],
                    num_idxs=128, num_idxs_reg=128, elem_size=G4)

            gather(0, ping)

            def step(t_expr, xp):
                # recurrence matmuls for both halves (read OLD hTa/hTb)
                zt = []
                for half in range(2):
                    for n in range(4):
                        g0 = 2048 * half + 512 * n
                        ps = zps.tile([128, 512], f32, tag="z", name="z")
                        for k in range(8):
                            src = hTa if k < 4 else hTb
                            nc.tensor.matmul(
                                ps[:], src[:, 128 * (k % 4):128 * (k % 4 + 1)],
                                whT_sb[:, k, g0:g0 + 512],
                                start=(k == 0), stop=(k == 7))
                        zt.append(ps)
                # gate tails per half (write NEW hTa/hTb)
                for half in range(2):
                    zf = work.tile([B, 2048], f32, tag="zf", name="zf")
                    for n in range(4):
                        nc.vector.tensor_add(
                            zf[:, 512 * n:512 * (n + 1)], zt[4 * half + n][:],
                            xp[:, 0, 2048 * half + 512 * n:
                               2048 * half + 512 * (n + 1)])
                    s = zf
                    nc.scalar.activation(s[:, 0:1536], zf[:, 0:1536],
                                         ACT.Sigmoid)
                    nc.scalar.activation(s[:, 1536:2048], zf[:, 1536:2048],
                                         ACT.Tanh)
                    hs = slice(512 * half, 512 * (half + 1))
                    ig = work.tile([B, 512], f32, tag="ig", name="ig")
                    nc.vector.tensor_mul(ig[:], s[:, 0:512], s[:, 1536:2048])
                    cf = work.tile([B, 512], f32, tag="cf", name="cf")
                    nc.vector.tensor_mul(cf[:], c_t[:, hs], s[:, 512:1024])
                    nc.vector.tensor_add(c_t[:, hs], cf[:], ig[:])
                    thc = work.tile([B, 512], f32, tag="thc", name="thc")
                    nc.scalar.activation(thc[:], c_t[:, hs], ACT.Tanh)
                    hh = work.tile([B, 512], f32, tag="hh", name="hh")
                    nc.vector.tensor_mul(hh[:], s[:, 1024:1536], thc[:])
                    nc.vector.scalar_tensor_tensor(
                        oacc[:, hs], hh[:], mask_sb[:, ds(t_expr, 1)],
                        oacc[:, hs],
                        mybir.AluOpType.mult, mybir.AluOpType.add)
                    tp = tps.tile([128, 512], f32, tag="tp", name="tp")
                    for j in range(4):
                        nc.tensor.transpose(tp[:, 128 * j:128 * (j + 1)],
                                            hh[:, 128 * j:128 * (j + 1)],
                                            ident[:])
                    nc.vector.tensor_copy((hTa if half == 0 else hTb)[:],
                                          tp[:])

            with tc.For_i(0, t_steps, 2) as tt:
                gather(tt + 1, pong)
                step(tt, ping)
                gather(tt + 2, ping)
                step(tt + 1, pong)

            # ---- final fc ----
            fps = fpsp.tile([1, B], f32, tag="fps", name="fps")
            for k4 in range(2):
                tpo = tps.tile([128, 512], f32, tag="tp", name=f"tpo{k4}")
                for j in range(4):
                    kk = 4 * k4 + j
                    nc.tensor.transpose(tpo[:, 128 * j:128 * (j + 1)],
                                        oacc[:, 128 * kk:128 * (kk + 1)],
                                        ident[:])
                oT = work.tile([128, 512], f32, tag="oT", name=f"oT{k4}")
                nc.vector.tensor_copy(oT[:], tpo[:])
                for j in range(4):
                    kk = 4 * k4 + j
                    nc.tensor.matmul(fps[:], fcw_sb[:, kk:kk + 1],
                                     oT[:, 128 * j:128 * (j + 1)],
                                     start=(kk == 0), stop=(kk == 7))
            ysb = work.tile([1, B], f32, tag="ysb", name="ysb")
            nc.vector.tensor_add(ysb[:], fps[:], fcb_sb[:])
            nc.sync.dma_start(y_d.ap(), ysb[:])

    nc.compile()
    _built[t_steps] = nc
    return nc


def _prep_inputs(x, lengths, emb, W_ii, W_hi, b_i, W_if, W_hf, b_f,
                 W_ig, W_hg, b_g, W_io, W_ho, b_o, fc_w, fc_b, t_steps):
    x = np.asarray(x).astype(np.int64)[:, :t_steps]
    lengths = np.asarray(lengths).astype(np.int64)
    emb = np.asarray(emb, dtype=np.float32).copy()
    emb[PAD_IDX] = 0.0

    # embT[p, ko, v] = emb[v, 128*ko + p]
    embT = np.zeros((128, 4, VPAD), dtype=ml_dtypes.bfloat16)
    embT[:, :, :V] = np.ascontiguousarray(
        emb.T.reshape(4, 128, V).transpose(1, 0, 2)).astype(ml_dtypes.bfloat16)

    # gate column permutation: [half][i f o g] x 512
    def permute_rows(Wi_, Wf_, Wg_, Wo_):
        blocks = []
        for half in range(2):
            hs = slice(512 * half, 512 * (half + 1))
            blocks += [np.asarray(Wi_)[hs], np.asarray(Wf_)[hs],
                       np.asarray(Wo_)[hs], np.asarray(Wg_)[hs]]
        return np.concatenate(blocks, axis=0)

    Wi_p = permute_rows(W_ii, W_if, W_ig, W_io)      # [4096, 512]
    Wh_p = permute_rows(W_hi, W_hf, W_hg, W_ho)      # [4096, 1024]
    b_p = permute_rows(b_i[:, None], b_f[:, None], b_g[:, None],
                       b_o[:, None]).ravel()         # [4096]

    # wiT[p, ke, g] = Wi_p[g, 128*ke + p]
    wiT = np.ascontiguousarray(
        Wi_p.T.reshape(4, 128, G4).transpose(1, 0, 2)).astype(
            ml_dtypes.bfloat16)
    whT = np.ascontiguousarray(
        Wh_p.T.reshape(8, 128, G4).transpose(1, 0, 2)).astype(
            ml_dtypes.bfloat16)
    brow = np.tile(b_p.astype(ml_dtypes.bfloat16)[None, :], (128, 1))

    # t-major token ids, SWDGE layout: per step a [16, 8] block, tiled x8
    xt = np.ascontiguousarray(x.T).astype(np.int16)     # [t, B]
    blocks = [xt[t].reshape(8, 16).T for t in range(t_steps)]
    blocks.append(np.zeros((16, 8), np.int16))          # overrun pad step
    idx16 = np.tile(np.concatenate(blocks, axis=1), (8, 1))

    maskv = (lengths[:, None] == (np.arange(t_steps)[None, :] + 1)).astype(
        np.float32)

    fc_w = np.asarray(fc_w, dtype=np.float32).reshape(O, H)
    fcw = np.ascontiguousarray(fc_w[0].reshape(8, 128).T)
    fcb = np.full((1, B), np.asarray(fc_b, np.float32).reshape(O)[0],
                  dtype=np.float32)

    return [{
        "embT": embT.reshape(128, 4 * VPAD),
        "wiT": wiT.reshape(128, 4 * G4),
        "whT": whT.reshape(128, 8 * G4),
        "brow": brow,
        "idx16": idx16,
        "maskv": maskv,
        "fcw": fcw,
        "fcb": fcb,
    }]


def kernel(**inputs):
    global LAST_EXEC_NS
    t_steps = int(os.environ.get("KERNEL_T", T_FULL))
    nc = _build(t_steps)
    in_maps = _prep_inputs(t_steps=t_steps, **inputs)
    res = run_bass_kernel_spmd(nc, in_maps, core_ids=[0])
    LAST_EXEC_NS = res.exec_time_ns
    y = np.asarray(res.results[0]["y"], dtype=np.float32).reshape(B)
    return y.reshape(B, O)


# revision 8
# speedup vs baseline: 4805.7820x; 1.1192x over previous
"""Trainium2 Bass kernel for the custom LSTM problem.

Strategy: single-core, collective-free. Per-step AllGather collectives cost
~13ms each under this runtime, so all cross-core schemes lose; one core runs
the full recurrence instead.

Phase A: build a projected embedding table P[v, :] = emb[v] @ Wi.T + b
([VPAD, 4G] bf16, in internal DRAM) — one 134-GFLOP GEMM instead of 275
GFLOP of per-step input projections.
Phase B: For_i hardware loop over T steps. Per step: dma_gather the 128
token rows of P (that step's xp, bias included), 64 N=512 matmuls for
h @ Wh.T accumulated on top in PSUM is not possible (PSUM too small), so
z = psum(h@Wh.T) + xp via DVE adds; gate order per 2048-wide half is
[i|f|o|g] so one Sigmoid covers 1536 cols and one Tanh the g block.
h is re-transposed each step with PE transposes (hT is the next step's
stationary operand). Hardware loop keeps the program ~500 instructions,
so bass build + NEFF compile stay fast.

Gate column permutation (host side): half h in {0,1}, order [i f o g]
within each half; h-dim (Wh columns, c, h state) stays in natural order.
"""

import os
import numpy as np
import ml_dtypes

import concourse.bass as bass
import concourse.mybir as mybir
import concourse.tile as tile
from concourse import bacc
from concourse.bass import ds
from concourse.bass_utils import run_bass_kernel_spmd
from concourse.masks import make_identity

V, E, H, B, T_FULL, O = 32000, 512, 1024, 128, 512, 1
VPAD = 32768
G4 = 4 * H          # 4096 gate columns
PAD_IDX = 0
NVT = VPAD // 128   # 256 vocab tiles

f32 = mybir.dt.float32
bf16 = mybir.dt.bfloat16
i16 = mybir.dt.int16
ACT = mybir.ActivationFunctionType

LAST_EXEC_NS = None
_built = {}


def _build(t_steps):
    if t_steps in _built:
        return _built[t_steps]
    assert t_steps % 2 == 0

    nc = bacc.Bacc("TRN2", target_bir_lowering=False, debug=False,
                   num_devices=1)

    embT_d = nc.dram_tensor("embT", [128, 4 * VPAD], bf16, kind="ExternalInput")
    wiT_d = nc.dram_tensor("wiT", [128, 4 * G4], bf16, kind="ExternalInput")
    whT_d = nc.dram_tensor("whT", [128, 8 * G4], bf16, kind="ExternalInput")
    brow_d = nc.dram_tensor("brow", [128, G4], bf16, kind="ExternalInput")
    idx_d = nc.dram_tensor("idx16", [128, 8 * t_steps + 8], i16,
                           kind="ExternalInput")
    mask_d = nc.dram_tensor("maskv", [B, t_steps], f32, kind="ExternalInput")
    fcw_d = nc.dram_tensor("fcw", [128, 8], f32, kind="ExternalInput")
    fcb_d = nc.dram_tensor("fcb", [1, B], f32, kind="ExternalInput")
    y_d = nc.dram_tensor("y", [1, B], f32, kind="ExternalOutput")

    with tile.TileContext(nc) as tc:
        with (
            tc.tile_pool(name="const", bufs=1) as constp,
            tc.tile_pool(name="state", bufs=1) as state,
            tc.tile_pool(name="work", bufs=2) as work,
            tc.tile_pool(name="emb", bufs=3) as embp,
            tc.tile_pool(name="ptcp", bufs=1) as ptcp,
            tc.tile_pool(name="zps", bufs=5, space="PSUM") as zps,
            tc.tile_pool(name="tps", bufs=2, space="PSUM") as tps,
            tc.tile_pool(name="fps", bufs=1, space="PSUM") as fpsp,
            tc.tile_pool(name="ptab", bufs=1, space="DRAM") as ptab,
        ):
            # ---- constants ----
            wiT_sb = constp.tile([128, 4, G4], bf16, name="wiT_sb")
            nc.sync.dma_start(
                wiT_sb[:], wiT_d.ap().rearrange("p (ko n) -> p ko n", ko=4))
            whT_sb = constp.tile([128, 8, G4], bf16, name="whT_sb")
            nc.sync.dma_start(
                whT_sb[:], whT_d.ap().rearrange("p (k n) -> p k n", k=8))
            brow_sb = constp.tile([128, G4], bf16, name="brow_sb")
            nc.sync.dma_start(brow_sb[:], brow_d.ap())
            idx_sb = constp.tile([128, 8 * t_steps + 8], i16, name="idx_sb")
            nc.sync.dma_start(idx_sb[:], idx_d.ap())
            mask_sb = constp.tile([B, t_steps], f32, name="mask_sb")
            nc.sync.dma_start(mask_sb[:], mask_d.ap())
            fcw_sb = constp.tile([128, 8], f32, name="fcw_sb")
            nc.sync.dma_start(fcw_sb[:], fcw_d.ap())
            fcb_sb = constp.tile([1, B], f32, name="fcb_sb")
            nc.sync.dma_start(fcb_sb[:], fcb_d.ap())
            ident = constp.tile([128, 128], f32, name="ident")
            make_identity(nc, ident[:])

            P_t = ptab.tile([VPAD, G4], bf16, name="P_t")

            embT_ap = embT_d.ap().rearrange("p (ko v) -> p ko v", ko=4)

            # ---- phase A: P = embT.T @ WiT + b ----
            def vtile(vb):
                et = embp.tile([128, 4, 128], bf16, tag="et", name="et")
                nc.sync.dma_start(et[:], embT_ap[:, :, ds(vb, 128)])
                pt = ptcp.tile([128, G4], bf16, tag="pt", name="pt")
                for n in range(8):
                    ps = zps.tile([128, 512], f32, tag="z", name=f"za{n}")
                    for ke in range(4):
                        nc.tensor.matmul(
                            ps[:], et[:, ke, :],
                            wiT_sb[:, ke, 512 * n:512 * (n + 1)],
                            start=(ke == 0), stop=(ke == 3))
                    nc.vector.tensor_add(
                        pt[:, 512 * n:512 * (n + 1)], ps[:],
                        brow_sb[:, 512 * n:512 * (n + 1)])
                nc.sync.dma_start(P_t[ds(vb, 128), :], pt[:])

            with tc.For_i(0, VPAD, 256) as vb:
                vtile(vb)
                vtile(vb + 128)

            # ---- phase B state ----
            c_t = state.tile([B, H], f32, name="c_t")
            nc.vector.memset(c_t[:], 0.0)
            oacc = state.tile([B, H], f32, name="oacc")
            nc.vector.memset(oacc[:], 0.0)
            hTa = state.tile([128, 512], bf16, name="hTa")
            nc.vector.memset(hTa[:], 0.0)
            hTb = state.tile([128, 512], bf16, name="hTb")
            nc.vector.memset(hTb[:], 0.0)
            ping = state.tile([128, 1, G4], bf16, name="ping")
            pong = state.tile([128, 1, G4], bf16, name="pong")

            def gather(step_expr, dst):
                nc.gpsimd.dma_gather(
                    out_ap=dst[:], in_ap=P_t[:],
                    idxs_ap=idx_sb[:, ds(step_expr * 8, 8)],
                    num_idxs=128, num_idxs_reg=128, elem_size=G4)

            gather(0, ping)

            def mm_banks(banks, ks, zt):
                # global bank n covers gate cols [512n, 512n+512)
                for n in banks:
                    if ks[0] == 0:
                        zt[n] = zps.tile([128, 512], f32, tag="z", name="z")
                    ps = zt[n]
                    for k in ks:
                        src = hTa if k < 4 else hTb
                        nc.tensor.matmul(
                            ps[:], src[:, 128 * (k % 4):128 * (k % 4 + 1)],
                            whT_sb[:, k, 512 * n:512 * (n + 1)],
                            start=(k == 0), stop=(k == 7))

            def tail(t_expr, xp, half, zt):
                zf = work.tile([B, 2048], f32, tag="zf", name="zf")
                for n in range(4):
                    bank = 4 * half + n
                    nc.vector.tensor_add(
                        zf[:, 512 * n:512 * (n + 1)], zt[bank][:],
                        xp[:, 0, 512 * bank:512 * (bank + 1)])
                s = zf
                # i,f first and g, so the c-chain starts before o's sigmoid
                nc.scalar.activation(s[:, 0:1024], zf[:, 0:1024], ACT.Sigmoid)
                nc.scalar.activation(s[:, 1536:2048], zf[:, 1536:2048],
                                     ACT.Tanh)
                nc.scalar.activation(s[:, 1024:1536], zf[:, 1024:1536],
                                     ACT.Sigmoid)
                hs = slice(512 * half, 512 * (half + 1))
                ig = work.tile([B, 512], f32, tag="ig", name="ig")
                nc.vector.tensor_mul(ig[:], s[:, 0:512], s[:, 1536:2048])
                cf = work.tile([B, 512], f32, tag="cf", name="cf")
                nc.vector.tensor_mul(cf[:], c_t[:, hs], s[:, 512:1024])
                nc.vector.tensor_add(c_t[:, hs], cf[:], ig[:])
                thc = work.tile([B, 512], f32, tag="thc", name="thc")
                nc.scalar.activation(thc[:], c_t[:, hs], ACT.Tanh)
                hh = work.tile([B, 512], f32, tag="hh", name="hh")
                nc.vector.tensor_mul(hh[:], s[:, 1024:1536], thc[:])
                nc.vector.scalar_tensor_tensor(
                    oacc[:, hs], hh[:], mask_sb[:, ds(t_expr, 1)],
                    oacc[:, hs],
                    mybir.AluOpType.mult, mybir.AluOpType.add)
                return hh

            def trcopy(half, hh):
                tp = tps.tile([128, 512], f32, tag="tp", name="tp")
                for j in range(4):
                    nc.tensor.transpose(tp[:, 128 * j:128 * (j + 1)],
                                        hh[:, 128 * j:128 * (j + 1)],
                                        ident[:])
                nc.vector.tensor_copy((hTa if half == 0 else hTb)[:], tp[:])

            with tc.For_i(0, t_steps, 2) as tt:
                gather(tt + 1, pong)
                zt0 = {}
                mm_banks(range(8), range(8), zt0)
                hhA = tail(tt, ping, 0, zt0)
                trcopy(0, hhA)
                hhB = tail(tt, ping, 1, zt0)
                zt1 = {}
                # t1's half-A banks can accumulate their hTa part while
                # step t0's half-B tail is still producing hTb
                mm_banks(range(4), range(4), zt1)
                trcopy(1, hhB)
                mm_banks(range(4), range(4, 8), zt1)
                gather(tt + 2, ping)
                mm_banks(range(4, 8), range(8), zt1)
                hhA = tail(tt + 1, pong, 0, zt1)
                trcopy(0, hhA)
                hhB = tail(tt + 1, pong, 1, zt1)
                trcopy(1, hhB)

            # ---- final fc ----
            fps = fpsp.tile([1, B], f32, tag="fps", name="fps")
            for k4 in range(2):
                tpo = tps.tile([128, 512], f32, tag="tp", name=f"tpo{k4}")
                for j in range(4):
                    kk = 4 * k4 + j
                    nc.tensor.transpose(tpo[:, 128 * j:128 * (j + 1)],
                                        oacc[:, 128 * kk:128 * (kk + 1)],
                                        ident[:])
                oT = work.tile([128, 512], f32, tag="oT", name=f"oT{k4}")
                nc.vector.tensor_copy(oT[:], tpo[:])
                for j in range(4):
                    kk = 4 * k4 + j
                    nc.tensor.matmul(fps[:], fcw_sb[:, kk:kk + 1],
                                     oT[:, 128 * j:128 * (j + 1)],
                                     start=(kk == 0), stop=(kk == 7))
            ysb = work.tile([1, B], f32, tag="ysb", name="ysb")
            nc.vector.tensor_add(ysb[:], fps[:], fcb_sb[:])
            nc.sync.dma_start(y_d.ap(), ysb[:])

    nc.compile()
    _built[t_steps] = nc
    return nc


def _prep_inputs(x, lengths, emb, W_ii, W_hi, b_i, W_if, W_hf, b_f,
                 W_ig, W_hg, b_g, W_io, W_ho, b_o, fc_w, fc_b, t_steps):
    x = np.asarray(x).astype(np.int64)[:, :t_steps]
    lengths = np.asarray(lengths).astype(np.int64)
    emb = np.asarray(emb, dtype=np.float32).copy()
    emb[PAD_IDX] = 0.0

    # embT[p, ko, v] = emb[v, 128*ko + p]
    embT = np.zeros((128, 4, VPAD), dtype=ml_dtypes.bfloat16)
    embT[:, :, :V] = np.ascontiguousarray(
        emb.T.reshape(4, 128, V).transpose(1, 0, 2)).astype(ml_dtypes.bfloat16)

    # gate column permutation: [half][i f o g] x 512
    def permute_rows(Wi_, Wf_, Wg_, Wo_):
        blocks = []
        for half in range(2):
            hs = slice(512 * half, 512 * (half + 1))
            blocks += [np.asarray(Wi_)[hs], np.asarray(Wf_)[hs],
                       np.asarray(Wo_)[hs], np.asarray(Wg_)[hs]]
        return np.concatenate(blocks, axis=0)

    Wi_p = permute_rows(W_ii, W_if, W_ig, W_io)      # [4096, 512]
    Wh_p = permute_rows(W_hi, W_hf, W_hg, W_ho)      # [4096, 1024]
    b_p = permute_rows(b_i[:, None], b_f[:, None], b_g[:, None],
                       b_o[:, None]).ravel()         # [4096]

    # wiT[p, ke, g] = Wi_p[g, 128*ke + p]
    wiT = np.ascontiguousarray(
        Wi_p.T.reshape(4, 128, G4).transpose(1, 0, 2)).astype(
            ml_dtypes.bfloat16)
    whT = np.ascontiguousarray(
        Wh_p.T.reshape(8, 128, G4).transpose(1, 0, 2)).astype(
            ml_dtypes.bfloat16)
    brow = np.tile(b_p.astype(ml_dtypes.bfloat16)[None, :], (128, 1))

    # t-major token ids, SWDGE layout: per step a [16, 8] block, tiled x8
    xt = np.ascontiguousarray(x.T).astype(np.int16)     # [t, B]
    blocks = [xt[t].reshape(8, 16).T for t in range(t_steps)]
    blocks.append(np.zeros((16, 8), np.int16))          # overrun pad step
    idx16 = np.tile(np.concatenate(blocks, axis=1), (8, 1))

    maskv = (lengths[:, None] == (np.arange(t_steps)[None, :] + 1)).astype(
        np.float32)

    fc_w = np.asarray(fc_w, dtype=np.float32).reshape(O, H)
    fcw = np.ascontiguousarray(fc_w[0].reshape(8, 128).T)
    fcb = np.full((1, B), np.asarray(fc_b, np.float32).reshape(O)[0],
                  dtype=np.float32)

    return [{
        "embT": embT.reshape(128, 4 * VPAD),
        "wiT": wiT.reshape(128, 4 * G4),
        "whT": whT.reshape(128, 8 * G4),
        "brow": brow,
        "idx16": idx16,
        "maskv": maskv,
        "fcw": fcw,
        "fcb": fcb,
    }]


def kernel(**inputs):
    global LAST_EXEC_NS
    t_steps = int(os.environ.get("KERNEL_T", T_FULL))
    nc = _build(t_steps)
    in_maps = _prep_inputs(t_steps=t_steps, **inputs)
    res = run_bass_kernel_spmd(nc, in_maps, core_ids=[0])
    LAST_EXEC_NS = res.exec_time_ns
    y = np.asarray(res.results[0]["y"], dtype=np.float32).reshape(B)
    return y.reshape(B, O)


# revision 9
# speedup vs baseline: 4879.2376x; 1.0153x over previous
"""Trainium2 Bass kernel for the custom LSTM problem.

Strategy: single-core, collective-free. Per-step AllGather collectives cost
~13ms each under this runtime, so all cross-core schemes lose; one core runs
the full recurrence instead.

Phase A: build a projected embedding table P[v, :] = emb[v] @ Wi.T + b
([VPAD, 4G] bf16, in internal DRAM) — one 134-GFLOP GEMM instead of 275
GFLOP of per-step input projections.
Phase B: For_i hardware loop over T steps. Per step: dma_gather the 128
token rows of P (that step's xp, bias included), 64 N=512 matmuls for
h @ Wh.T accumulated on top in PSUM is not possible (PSUM too small), so
z = psum(h@Wh.T) + xp via DVE adds; gate order per 2048-wide half is
[i|f|o|g] so one Sigmoid covers 1536 cols and one Tanh the g block.
h is re-transposed each step with PE transposes (hT is the next step's
stationary operand). Hardware loop keeps the program ~500 instructions,
so bass build + NEFF compile stay fast.

Gate column permutation (host side): half h in {0,1}, order [i f o g]
within each half; h-dim (Wh columns, c, h state) stays in natural order.
"""

import os
import numpy as np
import ml_dtypes

import concourse.bass as bass
import concourse.mybir as mybir
import concourse.tile as tile
from concourse import bacc
from concourse.bass import ds
from concourse.bass_utils import run_bass_kernel_spmd
from concourse.masks import make_identity

V, E, H, B, T_FULL, O = 32000, 512, 1024, 128, 512, 1
VPAD = 32768
G4 = 4 * H          # 4096 gate columns
PAD_IDX = 0
NVT = VPAD // 128   # 256 vocab tiles

f32 = mybir.dt.float32
bf16 = mybir.dt.bfloat16
i16 = mybir.dt.int16
ACT = mybir.ActivationFunctionType

LAST_EXEC_NS = None
_built = {}


def _build(t_steps):
    if t_steps in _built:
        return _built[t_steps]
    assert t_steps % 2 == 0

    nc = bacc.Bacc("TRN2", target_bir_lowering=False, debug=False,
                   num_devices=1)

    embT_d = nc.dram_tensor("embT", [128, 4 * VPAD], bf16, kind="ExternalInput")
    wiT_d = nc.dram_tensor("wiT", [128, 4 * G4], bf16, kind="ExternalInput")
    whT_d = nc.dram_tensor("whT", [128, 8 * G4], bf16, kind="ExternalInput")
    brow_d = nc.dram_tensor("brow", [128, G4], bf16, kind="ExternalInput")
    idx_d = nc.dram_tensor("idx16", [128, 8 * t_steps + 8], i16,
                           kind="ExternalInput")
    mask_d = nc.dram_tensor("maskv", [B, t_steps], f32, kind="ExternalInput")
    fcw_d = nc.dram_tensor("fcw", [128, 8], f32, kind="ExternalInput")
    fcb_d = nc.dram_tensor("fcb", [1, B], f32, kind="ExternalInput")
    y_d = nc.dram_tensor("y", [1, B], f32, kind="ExternalOutput")

    with tile.TileContext(nc) as tc:
        with (
            tc.tile_pool(name="const", bufs=1) as constp,
            tc.tile_pool(name="state", bufs=1) as state,
            tc.tile_pool(name="work", bufs=2) as work,
            tc.tile_pool(name="emb", bufs=3) as embp,
            tc.tile_pool(name="ptcp", bufs=1) as ptcp,
            tc.tile_pool(name="zps", bufs=5, space="PSUM") as zps,
            tc.tile_pool(name="tps", bufs=2, space="PSUM") as tps,
            tc.tile_pool(name="fps", bufs=1, space="PSUM") as fpsp,
            tc.tile_pool(name="ptab", bufs=1, space="DRAM") as ptab,
        ):
            # ---- constants ----
            wiT_sb = constp.tile([128, 4, G4], bf16, name="wiT_sb")
            nc.sync.dma_start(
                wiT_sb[:], wiT_d.ap().rearrange("p (ko n) -> p ko n", ko=4))
            whT_sb = constp.tile([128, 8, G4], bf16, name="whT_sb")
            nc.sync.dma_start(
                whT_sb[:], whT_d.ap().rearrange("p (k n) -> p k n", k=8))
            brow_sb = constp.tile([128, G4], bf16, name="brow_sb")
            nc.sync.dma_start(brow_sb[:], brow_d.ap())
            idx_sb = constp.tile([128, 8 * t_steps + 8], i16, name="idx_sb")
            nc.sync.dma_start(idx_sb[:], idx_d.ap())
            mask_sb = constp.tile([B, t_steps], f32, name="mask_sb")
            nc.sync.dma_start(mask_sb[:], mask_d.ap())
            fcw_sb = constp.tile([128, 8], f32, name="fcw_sb")
            nc.sync.dma_start(fcw_sb[:], fcw_d.ap())
            fcb_sb = constp.tile([1, B], f32, name="fcb_sb")
            nc.sync.dma_start(fcb_sb[:], fcb_d.ap())
            ident = constp.tile([128, 128], f32, name="ident")
            make_identity(nc, ident[:])

            P_t = ptab.tile([VPAD, G4], bf16, name="P_t")

            embT_ap = embT_d.ap().rearrange("p (ko v) -> p ko v", ko=4)

            # ---- phase A: P = embT.T @ WiT + b ----
            def vtile(vb):
                et = embp.tile([128, 4, 128], bf16, tag="et", name="et")
                nc.sync.dma_start(et[:], embT_ap[:, :, ds(vb, 128)])
                pt = ptcp.tile([128, G4], bf16, tag="pt", name="pt")
                for n in range(8):
                    ps = zps.tile([128, 512], f32, tag="z", name=f"za{n}")
                    for ke in range(4):
                        nc.tensor.matmul(
                            ps[:], et[:, ke, :],
                            wiT_sb[:, ke, 512 * n:512 * (n + 1)],
                            start=(ke == 0), stop=(ke == 3))
                    nc.vector.tensor_add(
                        pt[:, 512 * n:512 * (n + 1)], ps[:],
                        brow_sb[:, 512 * n:512 * (n + 1)])
                nc.sync.dma_start(P_t[ds(vb, 128), :], pt[:])

            with tc.For_i(0, VPAD, 256) as vb:
                vtile(vb)
                vtile(vb + 128)

            # ---- phase B state ----
            c_t = state.tile([B, H], f32, name="c_t")
            nc.vector.memset(c_t[:], 0.0)
            oacc = state.tile([B, H], f32, name="oacc")
            nc.vector.memset(oacc[:], 0.0)
            hTa = state.tile([128, 512], bf16, name="hTa")
            nc.vector.memset(hTa[:], 0.0)
            hTb = state.tile([128, 512], bf16, name="hTb")
            nc.vector.memset(hTb[:], 0.0)
            ping = state.tile([128, 1, G4], bf16, name="ping")
            pong = state.tile([128, 1, G4], bf16, name="pong")

            def gather(step_expr, dst):
                nc.gpsimd.dma_gather(
                    out_ap=dst[:], in_ap=P_t[:],
                    idxs_ap=idx_sb[:, ds(step_expr * 8, 8)],
                    num_idxs=128, num_idxs_reg=128, elem_size=G4)

            gather(0, ping)

            def step(t_expr, xp):
                # recurrence matmuls for both halves (read OLD hTa/hTb)
                zt = []
                for half in range(2):
                    for n in range(4):
                        g0 = 2048 * half + 512 * n
                        ps = zps.tile([128, 512], f32, tag="z", name="z")
                        for k in range(8):
                            src = hTa if k < 4 else hTb
                            nc.tensor.matmul(
                                ps[:], src[:, 128 * (k % 4):128 * (k % 4 + 1)],
                                whT_sb[:, k, g0:g0 + 512],
                                start=(k == 0), stop=(k == 7))
                        zt.append(ps)
                # gate tails per half (write NEW hTa/hTb)
                for half in range(2):
                    zf = work.tile([B, 2048], f32, tag="zf", name="zf")
                    for n in range(4):
                        nc.vector.tensor_add(
                            zf[:, 512 * n:512 * (n + 1)], zt[4 * half + n][:],
                            xp[:, 0, 2048 * half + 512 * n:
                               2048 * half + 512 * (n + 1)])
                    s = zf
                    nc.scalar.activation(s[:, 0:1536], zf[:, 0:1536],
                                         ACT.Sigmoid)
                    nc.scalar.activation(s[:, 1536:2048], zf[:, 1536:2048],
                                         ACT.Tanh)
                    hs = slice(512 * half, 512 * (half + 1))
                    ig = work.tile([B, 512], f32, tag="ig", name="ig")
                    nc.vector.tensor_mul(ig[:], s[:, 0:512], s[:, 1536:2048])
                    cf = work.tile([B, 512], f32, tag="cf", name="cf")
                    nc.vector.tensor_mul(cf[:], c_t[:, hs], s[:, 512:1024])
                    nc.vector.tensor_add(c_t[:, hs], cf[:], ig[:])
                    thc = work.tile([B, 512], f32, tag="thc", name="thc")
                    nc.scalar.activation(thc[:], c_t[:, hs], ACT.Tanh)
                    hh = work.tile([B, 512], f32, tag="hh", name="hh")
                    nc.vector.tensor_mul(hh[:], s[:, 1024:1536], thc[:])
                    nc.vector.scalar_tensor_tensor(
                        oacc[:, hs], hh[:], mask_sb[:, ds(t_expr, 1)],
                        oacc[:, hs],
                        mybir.AluOpType.mult, mybir.AluOpType.add)
                    tp = tps.tile([128, 512], f32, tag="tp", name="tp")
                    for j in range(4):
                        nc.tensor.transpose(tp[:, 128 * j:128 * (j + 1)],
                                            hh[:, 128 * j:128 * (j + 1)],
                                            ident[:])
                    nc.vector.tensor_copy((hTa if half == 0 else hTb)[:],
                                          tp[:])

            with tc.For_i(0, t_steps, 2) as tt:
                gather(tt + 1, pong)
                step(tt, ping)
                gather(tt + 2, ping)
                step(tt + 1, pong)

            # ---- final fc ----
            fps = fpsp.tile([1, B], f32, tag="fps", name="fps")
            for k4 in range(2):
                tpo = tps.tile([128, 512], f32, tag="tp", name=f"tpo{k4}")
                for j in range(4):
                    kk = 4 * k4 + j
                    nc.tensor.transpose(tpo[:, 128 * j:128 * (j + 1)],
                                        oacc[:, 128 * kk:128 * (kk + 1)],
                                        ident[:])
                oT = work.tile([128, 512], f32, tag="oT", name=f"oT{k4}")
                nc.vector.tensor_copy(oT[:], tpo[:])
                for j in range(4):
                    kk = 4 * k4 + j
                    nc.tensor.matmul(fps[:], fcw_sb[:, kk:kk + 1],
                                     oT[:, 128 * j:128 * (j + 1)],
                                     start=(kk == 0), stop=(kk == 7))
            ysb = work.tile([1, B], f32, tag="ysb", name="ysb")
            nc.vector.tensor_add(ysb[:], fps[:], fcb_sb[:])
            nc.sync.dma_start(y_d.ap(), ysb[:])

    nc.compile()
    _built[t_steps] = nc
    return nc


def _prep_inputs(x, lengths, emb, W_ii, W_hi, b_i, W_if, W_hf, b_f,
                 W_ig, W_hg, b_g, W_io, W_ho, b_o, fc_w, fc_b, t_steps):
    x = np.asarray(x).astype(np.int64)[:, :t_steps]
    lengths = np.asarray(lengths).astype(np.int64)
    emb = np.asarray(emb, dtype=np.float32).copy()
    emb[PAD_IDX] = 0.0

    # embT[p, ko, v] = emb[v, 128*ko + p]
    embT = np.zeros((128, 4, VPAD), dtype=ml_dtypes.bfloat16)
    embT[:, :, :V] = np.ascontiguousarray(
        emb.T.reshape(4, 128, V).transpose(1, 0, 2)).astype(ml_dtypes.bfloat16)

    # gate column permutation: [half][i f o g] x 512
    def permute_rows(Wi_, Wf_, Wg_, Wo_):
        blocks = []
        for half in range(2):
            hs = slice(512 * half, 512 * (half + 1))
            blocks += [np.asarray(Wi_)[hs], np.asarray(Wf_)[hs],
                       np.asarray(Wo_)[hs], np.asarray(Wg_)[hs]]
        return np.concatenate(blocks, axis=0)

    Wi_p = permute_rows(W_ii, W_if, W_ig, W_io)      # [4096, 512]
    Wh_p = permute_rows(W_hi, W_hf, W_hg, W_ho)      # [4096, 1024]
    b_p = permute_rows(b_i[:, None], b_f[:, None], b_g[:, None],
                       b_o[:, None]).ravel()         # [4096]

    # wiT[p, ke, g] = Wi_p[g, 128*ke + p]
    wiT = np.ascontiguousarray(
        Wi_p.T.reshape(4, 128, G4).transpose(1, 0, 2)).astype(
            ml_dtypes.bfloat16)
    whT = np.ascontiguousarray(
        Wh_p.T.reshape(8, 128, G4).transpose(1, 0, 2)).astype(
            ml_dtypes.bfloat16)
    brow = np.tile(b_p.astype(ml_dtypes.bfloat16)[None, :], (128, 1))

    # t-major token ids, SWDGE layout: per step a [16, 8] block, tiled x8
    xt = np.ascontiguousarray(x.T).astype(np.int16)     # [t, B]
    blocks = [xt[t].reshape(8, 16).T for t in range(t_steps)]
    blocks.append(np.zeros((16, 8), np.int16))          # overrun pad step
    idx16 = np.tile(np.concatenate(blocks, axis=1), (8, 1))

    maskv = (lengths[:, None] == (np.arange(t_steps)[None, :] + 1)).astype(
        np.float32)

    fc_w = np.asarray(fc_w, dtype=np.float32).reshape(O, H)
    fcw = np.ascontiguousarray(fc_w[0].reshape(8, 128).T)
    fcb = np.full((1, B), np.asarray(fc_b, np.float32).reshape(O)[0],
                  dtype=np.float32)

    return [{
        "embT": embT.reshape(128, 4 * VPAD),
        "wiT": wiT.reshape(128, 4 * G4),
        "whT": whT.reshape(128, 8 * G4),
        "brow": brow,
        "idx16": idx16,
        "maskv": maskv,
        "fcw": fcw,
        "fcb": fcb,
    }]


def kernel(**inputs):
    global LAST_EXEC_NS
    t_steps = int(os.environ.get("KERNEL_T", T_FULL))
    nc = _build(t_steps)
    in_maps = _prep_inputs(t_steps=t_steps, **inputs)
    res = run_bass_kernel_spmd(nc, in_maps, core_ids=[0])
    LAST_EXEC_NS = res.exec_time_ns
    y = np.asarray(res.results[0]["y"], dtype=np.float32).reshape(B)
    return y.reshape(B, O)


# revision 10
# speedup vs baseline: 5074.6281x; 1.0400x over previous
"""Trainium2 Bass kernel for the custom LSTM problem.

Strategy: single-core, collective-free. Per-step AllGather collectives cost
~13ms each under this runtime, so all cross-core schemes lose; one core runs
the full recurrence instead.

Phase A: build a projected embedding table P[v, :] = emb[v] @ Wi.T + b
([VPAD, 4G] bf16, in internal DRAM) — one 134-GFLOP GEMM instead of 275
GFLOP of per-step input projections.
Phase B: For_i hardware loop over T steps. Per step: dma_gather the 128
token rows of P (that step's xp, bias included), 64 N=512 matmuls for
h @ Wh.T accumulated on top in PSUM is not possible (PSUM too small), so
z = psum(h@Wh.T) + xp via DVE adds; gate order per 2048-wide half is
[i|f|o|g] so one Sigmoid covers 1536 cols and one Tanh the g block.
h is re-transposed each step with PE transposes (hT is the next step's
stationary operand). Hardware loop keeps the program ~500 instructions,
so bass build + NEFF compile stay fast.

Gate column permutation (host side): half h in {0,1}, order [i f o g]
within each half; h-dim (Wh columns, c, h state) stays in natural order.
"""

import os
import numpy as np
import ml_dtypes

import concourse.bass as bass
import concourse.mybir as mybir
import concourse.tile as tile
from concourse import bacc
from concourse.bass import ds
from concourse.bass_utils import run_bass_kernel_spmd
from concourse.masks import make_identity

V, E, H, B, T_FULL, O = 32000, 512, 1024, 128, 512, 1
VPAD = 32768
G4 = 4 * H          # 4096 gate columns
PAD_IDX = 0
NVT = VPAD // 128   # 256 vocab tiles

f32 = mybir.dt.float32
bf16 = mybir.dt.bfloat16
i16 = mybir.dt.int16
ACT = mybir.ActivationFunctionType

LAST_EXEC_NS = None
_built = {}


def _build(t_steps):
    if t_steps in _built:
        return _built[t_steps]
    assert t_steps % 2 == 0

    nc = bacc.Bacc("TRN2", target_bir_lowering=False, debug=False,
                   num_devices=1)

    embT_d = nc.dram_tensor("embT", [128, 4 * VPAD], bf16, kind="ExternalInput")
    wiT_d = nc.dram_tensor("wiT", [128, 4 * G4], bf16, kind="ExternalInput")
    whT_d = nc.dram_tensor("whT", [128, 8 * G4], bf16, kind="ExternalInput")
    brow_d = nc.dram_tensor("brow", [128, G4], bf16, kind="ExternalInput")
    idx_d = nc.dram_tensor("idx16", [128, 8 * t_steps + 8], i16,
                           kind="ExternalInput")
    mask_d = nc.dram_tensor("maskv", [B, t_steps], f32, kind="ExternalInput")
    fcw_d = nc.dram_tensor("fcw", [128, 8], f32, kind="ExternalInput")
    fcb_d = nc.dram_tensor("fcb", [1, B], f32, kind="ExternalInput")
    y_d = nc.dram_tensor("y", [1, B], f32, kind="ExternalOutput")

    with tile.TileContext(nc) as tc:
        with (
            tc.tile_pool(name="const", bufs=1) as constp,
            tc.tile_pool(name="state", bufs=1) as state,
            tc.tile_pool(name="work", bufs=2) as work,
            tc.tile_pool(name="emb", bufs=3) as embp,
            tc.tile_pool(name="ptcp", bufs=1) as ptcp,
            tc.tile_pool(name="zps", bufs=5, space="PSUM") as zps,
            tc.tile_pool(name="tps", bufs=2, space="PSUM") as tps,
            tc.tile_pool(name="fps", bufs=1, space="PSUM") as fpsp,
            tc.tile_pool(name="ptab", bufs=1, space="DRAM") as ptab,
        ):
            # ---- constants ----
            wiT_sb = constp.tile([128, 4, G4], bf16, name="wiT_sb")
            nc.sync.dma_start(
                wiT_sb[:], wiT_d.ap().rearrange("p (ko n) -> p ko n", ko=4))
            whT_sb = constp.tile([128, 8, G4], bf16, name="whT_sb")
            nc.sync.dma_start(
                whT_sb[:], whT_d.ap().rearrange("p (k n) -> p k n", k=8))
            brow_sb = constp.tile([128, G4], bf16, name="brow_sb")
            nc.sync.dma_start(brow_sb[:], brow_d.ap())
            idx_sb = constp.tile([128, 8 * t_steps + 8], i16, name="idx_sb")
            nc.sync.dma_start(idx_sb[:], idx_d.ap())
            mask_sb = constp.tile([B, t_steps], f32, name="mask_sb")
            nc.sync.dma_start(mask_sb[:], mask_d.ap())
            fcw_sb = constp.tile([128, 8], f32, name="fcw_sb")
            nc.sync.dma_start(fcw_sb[:], fcw_d.ap())
            fcb_sb = constp.tile([1, B], f32, name="fcb_sb")
            nc.sync.dma_start(fcb_sb[:], fcb_d.ap())
            ident = constp.tile([128, 128], f32, name="ident")
            make_identity(nc, ident[:])

            P_t = ptab.tile([VPAD, G4], bf16, name="P_t")

            embT_ap = embT_d.ap().rearrange("p (ko v) -> p ko v", ko=4)

            # ---- phase A: P = embT.T @ WiT + b ----
            def vtile(vb):
                et = embp.tile([128, 4, 128], bf16, tag="et", name="et")
                nc.sync.dma_start(et[:], embT_ap[:, :, ds(vb, 128)])
                pt = ptcp.tile([128, G4], bf16, tag="pt", name="pt")
                for n in range(8):
                    ps = zps.tile([128, 512], f32, tag="z", name=f"za{n}")
                    for ke in range(4):
                        nc.tensor.matmul(
                            ps[:], et[:, ke, :],
                            wiT_sb[:, ke, 512 * n:512 * (n + 1)],
                            start=(ke == 0), stop=(ke == 3))
                    nc.vector.tensor_add(
                        pt[:, 512 * n:512 * (n + 1)], ps[:],
                        brow_sb[:, 512 * n:512 * (n + 1)])
                nc.sync.dma_start(P_t[ds(vb, 128), :], pt[:])

            with tc.For_i(0, VPAD, 256) as vb:
                vtile(vb)
                vtile(vb + 128)

            # ---- phase B state ----
            c_t = state.tile([B, H], f32, name="c_t")
            nc.vector.memset(c_t[:], 0.0)
            oacc = state.tile([B, H], f32, name="oacc")
            nc.vector.memset(oacc[:], 0.0)
            hTa = state.tile([128, 512], bf16, name="hTa")
            nc.vector.memset(hTa[:], 0.0)
            hTb = state.tile([128, 512], bf16, name="hTb")
            nc.vector.memset(hTb[:], 0.0)
            ping = state.tile([128, 1, G4], bf16, name="ping")
            pong = state.tile([128, 1, G4], bf16, name="pong")

            def gather(step_expr, dst):
                nc.gpsimd.dma_gather(
                    out_ap=dst[:], in_ap=P_t[:],
                    idxs_ap=idx_sb[:, ds(step_expr * 8, 8)],
                    num_idxs=128, num_idxs_reg=128, elem_size=G4)

            gather(0, ping)

            def mm_banks(banks, ks, zt):
                # global bank n covers gate cols [512n, 512n+512)
                for n in banks:
                    if ks[0] == 0:
                        zt[n] = zps.tile([128, 512], f32, tag="z", name="z")
                    ps = zt[n]
                    for k in ks:
                        src = hTa if k < 4 else hTb
                        nc.tensor.matmul(
                            ps[:], src[:, 128 * (k % 4):128 * (k % 4 + 1)],
                            whT_sb[:, k, 512 * n:512 * (n + 1)],
                            start=(k == 0), stop=(k == 7))

            def tail(t_expr, xp, half, zt):
                zf = work.tile([B, 2048], f32, tag="zf", name="zf")
                for n in range(4):
                    bank = 4 * half + n
                    nc.vector.tensor_add(
                        zf[:, 512 * n:512 * (n + 1)], zt[bank][:],
                        xp[:, 0, 512 * bank:512 * (bank + 1)])
                s = zf
                # i,f first and g, so the c-chain starts before o's sigmoid
                nc.scalar.activation(s[:, 0:1024], zf[:, 0:1024], ACT.Sigmoid)
                nc.scalar.activation(s[:, 1536:2048], zf[:, 1536:2048],
                                     ACT.Tanh)
                nc.scalar.activation(s[:, 1024:1536], zf[:, 1024:1536],
                                     ACT.Sigmoid)
                hs = slice(512 * half, 512 * (half + 1))
                ig = work.tile([B, 512], f32, tag="ig", name="ig")
                nc.vector.tensor_mul(ig[:], s[:, 0:512], s[:, 1536:2048])
                cf = work.tile([B, 512], f32, tag="cf", name="cf")
                nc.vector.tensor_mul(cf[:], c_t[:, hs], s[:, 512:1024])
                nc.vector.tensor_add(c_t[:, hs], cf[:], ig[:])
                thc = work.tile([B, 512], f32, tag="thc", name="thc")
                nc.scalar.activation(thc[:], c_t[:, hs], ACT.Tanh)
                hh = work.tile([B, 512], f32, tag="hh", name="hh")
                nc.vector.tensor_mul(hh[:], s[:, 1024:1536], thc[:])
                nc.vector.scalar_tensor_tensor(
                    oacc[:, hs], hh[:], mask_sb[:, ds(t_expr, 1)],
                    oacc[:, hs],
                    mybir.AluOpType.mult, mybir.AluOpType.add)
                return hh

            def trcopy(half, hh):
                tp = tps.tile([128, 512], f32, tag="tp", name="tp")
                for j in range(4):
                    nc.tensor.transpose(tp[:, 128 * j:128 * (j + 1)],
                                        hh[:, 128 * j:128 * (j + 1)],
                                        ident[:])
                nc.vector.tensor_copy((hTa if half == 0 else hTb)[:], tp[:])

            with tc.For_i(0, t_steps, 2) as tt:
                gather(tt + 1, pong)
                zt0 = {}
                mm_banks(range(8), range(8), zt0)
                hhA = tail(tt, ping, 0, zt0)
                trcopy(0, hhA)
                hhB = tail(tt, ping, 1, zt0)
                zt1 = {}
                # t1's half-A banks can accumulate their hTa part while
                # step t0's half-B tail is still producing hTb
                mm_banks(range(4), range(4), zt1)
                trcopy(1, hhB)
                mm_banks(range(4), range(4, 8), zt1)
                gather(tt + 2, ping)
                mm_banks(range(4, 8), range(8), zt1)
                hhA = tail(tt + 1, pong, 0, zt1)
                trcopy(0, hhA)
                hhB = tail(tt + 1, pong, 1, zt1)
                trcopy(1, hhB)

            # ---- final fc ----
            fps = fpsp.tile([1, B], f32, tag="fps", name="fps")
            for k4 in range(2):
                tpo = tps.tile([128, 512], f32, tag="tp", name=f"tpo{k4}")
                for j in range(4):
                    kk = 4 * k4 + j
                    nc.tensor.transpose(tpo[:, 128 * j:128 * (j + 1)],
                                        oacc[:, 128 * kk:128 * (kk + 1)],
                                        ident[:])
                oT = work.tile([128, 512], f32, tag="oT", name=f"oT{k4}")
                nc.vector.tensor_copy(oT[:], tpo[:])
                for j in range(4):
                    kk = 4 * k4 + j
                    nc.tensor.matmul(fps[:], fcw_sb[:, kk:kk + 1],
                                     oT[:, 128 * j:128 * (j + 1)],
                                     start=(kk == 0), stop=(kk == 7))
            ysb = work.tile([1, B], f32, tag="ysb", name="ysb")
            nc.vector.tensor_add(ysb[:], fps[:], fcb_sb[:])
            nc.sync.dma_start(y_d.ap(), ysb[:])

    nc.compile()
    _built[t_steps] = nc
    return nc


def _prep_inputs(x, lengths, emb, W_ii, W_hi, b_i, W_if, W_hf, b_f,
                 W_ig, W_hg, b_g, W_io, W_ho, b_o, fc_w, fc_b, t_steps):
    x = np.asarray(x).astype(np.int64)[:, :t_steps]
    lengths = np.asarray(lengths).astype(np.int64)
    emb = np.asarray(emb, dtype=np.float32).copy()
    emb[PAD_IDX] = 0.0

    # embT[p, ko, v] = emb[v, 128*ko + p]
    embT = np.zeros((128, 4, VPAD), dtype=ml_dtypes.bfloat16)
    embT[:, :, :V] = np.ascontiguousarray(
        emb.T.reshape(4, 128, V).transpose(1, 0, 2)).astype(ml_dtypes.bfloat16)

    # gate column permutation: [half][i f o g] x 512
    def permute_rows(Wi_, Wf_, Wg_, Wo_):
        blocks = []
        for half in range(2):
            hs = slice(512 * half, 512 * (half + 1))
            blocks += [np.asarray(Wi_)[hs], np.asarray(Wf_)[hs],
                       np.asarray(Wo_)[hs], np.asarray(Wg_)[hs]]
        return np.concatenate(blocks, axis=0)

    Wi_p = permute_rows(W_ii, W_if, W_ig, W_io)      # [4096, 512]
    Wh_p = permute_rows(W_hi, W_hf, W_hg, W_ho)      # [4096, 1024]
    b_p = permute_rows(b_i[:, None], b_f[:, None], b_g[:, None],
                       b_o[:, None]).ravel()         # [4096]

    # wiT[p, ke, g] = Wi_p[g, 128*ke + p]
    wiT = np.ascontiguousarray(
        Wi_p.T.reshape(4, 128, G4).transpose(1, 0, 2)).astype(
            ml_dtypes.bfloat16)
    whT = np.ascontiguousarray(
        Wh_p.T.reshape(8, 128, G4).transpose(1, 0, 2)).astype(
            ml_dtypes.bfloat16)
    brow = np.tile(b_p.astype(ml_dtypes.bfloat16)[None, :], (128, 1))

    # t-major token ids, SWDGE layout: per step a [16, 8] block, tiled x8
    xt = np.ascontiguousarray(x.T).astype(np.int16)     # [t, B]
    blocks = [xt[t].reshape(8, 16).T for t in range(t_steps)]
    blocks.append(np.zeros((16, 8), np.int16))          # overrun pad step
    idx16 = np.tile(np.concatenate(blocks, axis=1), (8, 1))

    maskv = (lengths[:, None] == (np.arange(t_steps)[None, :] + 1)).astype(
        np.float32)

    fc_w = np.asarray(fc_w, dtype=np.float32).reshape(O, H)
    fcw = np.ascontiguousarray(fc_w[0].reshape(8, 128).T)
    fcb = np.full((1, B), np.asarray(fc_b, np.float32).reshape(O)[0],
                  dtype=np.float32)

    return [{
        "embT": embT.reshape(128, 4 * VPAD),
        "wiT": wiT.reshape(128, 4 * G4),
        "whT": whT.reshape(128, 8 * G4),
        "brow": brow,
        "idx16": idx16,
        "maskv": maskv,
        "fcw": fcw,
        "fcb": fcb,
    }]


def kernel(**inputs):
    global LAST_EXEC_NS
    t_steps = int(os.environ.get("KERNEL_T", T_FULL))
    nc = _build(t_steps)
    in_maps = _prep_inputs(t_steps=t_steps, **inputs)
    res = run_bass_kernel_spmd(nc, in_maps, core_ids=[0])
    LAST_EXEC_NS = res.exec_time_ns
    y = np.asarray(res.results[0]["y"], dtype=np.float32).reshape(B)
    return y.reshape(B, O)
